# revision 60
# baseline (speedup 1.0000x reference)
import sys

if "/opt/trn_rl_repo" not in sys.path:
    sys.path.insert(0, "/opt/trn_rl_repo")

import numpy as np

B, T, C = 2, 2048, 2048
H, H_KV = 16, 8
D = C // H  # 128
NCORES = 8
HL = H // NCORES  # 2 local query heads per core; 1 kv head per core

SCALE = 0.08838834764831845  # 1/sqrt(128)


def build_nc(b=B, t=T, c=C, mmdt="f16"):
    """Per-core Bass program, merged proj/attention/output pipeline.

    Same program on all 8 cores; the sharding lives in the input data each
    core receives (its 2 query heads + 1 kv head slice of Wq/Wk/Wv, and the
    matching 256-row slice of Wp; partial y outputs summed on host)."""
    import concourse.bass as bass  # noqa: F401
    import concourse.mybir as mybir
    import concourse.tile as tile
    from concourse import bacc

    f32 = mybir.dt.float32
    f16 = {
        "f32r": mybir.dt.float32r,
        "bf16": mybir.dt.bfloat16,
        "f16": mybir.dt.float16,
    }[mmdt]
    EXP = mybir.ActivationFunctionType.Exp

    ncb = c // 128  # contraction blocks for projections
    nt = t // 512  # 512-wide t tiles
    njb = 512 // 128  # k-blocks per 512-wide q tile

    nc = bacc.Bacc("TRN2", target_bir_lowering=False, debug=False)

    xT = nc.dram_tensor("xT", [b, c, t], f16, kind="ExternalInput")
    wq = nc.dram_tensor("wq", [c, HL * D], f16, kind="ExternalInput")
    wk = nc.dram_tensor("wk", [c, D], f16, kind="ExternalInput")
    wv = nc.dram_tensor("wv", [c, D], f16, kind="ExternalInput")
    wp = nc.dram_tensor("wp", [HL * D, c], f16, kind="ExternalInput")
    cos2 = nc.dram_tensor("cos2", [128, t], f16, kind="ExternalInput")
    sin2 = nc.dram_tensor("sin2", [128, t], f16, kind="ExternalInput")
    maskf = nc.dram_tensor("maskf", [128, 128], f16, kind="ExternalInput")
    onesv = nc.dram_tensor("onesv", [128, 1], f16, kind="ExternalInput")
    ident = nc.dram_tensor("ident", [128, 128], f16, kind="ExternalInput")
    y = nc.dram_tensor("y", [b, t, c], f16, kind="ExternalOutput")

    swap_mask = [i ^ 1 for i in range(32)]

    with tile.TileContext(nc) as tc:
        with (
            tc.tile_pool(name="wts", bufs=1) as wpool,
            tc.tile_pool(name="data", bufs=1) as dpool,
            tc.tile_pool(name="work", bufs=2) as wkp,
            tc.tile_pool(name="psum", bufs=1, space="PSUM") as pp,
        ):
            # ---- preamble DMA order is the startup critical path (one
            # shared DMA engine): small tables + first weight chunks first,
            # then x-tile prefetch, then the remaining weight chunks. ----
            nw = max(ncb // 4, 1)

            wq_sbs, wk_sbs, wv_sbs = [], [], []

            def load_w_chunk(wi):
                cbs = slice(wi * nw * 128, (wi + 1) * nw * 128)
                wq_i = wpool.tile([128, nw * HL * D], f16, name=f"wq{wi}")
                nc.sync.dma_start(
                    wq_i[:].rearrange("p (cb d) -> p cb d", d=HL * D),
                    wq[cbs, :].rearrange("(cb p) d -> p cb d", p=128),
                )
                wq_sbs.append(wq_i)
                wk_i = wpool.tile([128, nw * D], f16, name=f"wk{wi}")
                nc.sync.dma_start(
                    wk_i[:].rearrange("p (cb d) -> p cb d", d=D),
                    wk[cbs, :].rearrange("(cb p) d -> p cb d", p=128),
                )
                wk_sbs.append(wk_i)
                wv_i = wpool.tile([128, nw * D], f16, name=f"wv{wi}")
                nc.sync.dma_start(
                    wv_i[:].rearrange("p (cb d) -> p cb d", d=D),
                    wv[cbs, :].rearrange("(cb p) d -> p cb d", p=128),
                )
                wv_sbs.append(wv_i)

            xt_pre = {}

            def prefetch_xt(cb0, cb1):
                for cb in range(cb0, cb1):
                    xtp = wkp.tile([128, 512], f16, tag="xt", bufs=32, name=f"xtp{cb}")
                    nc.sync.dma_start(xtp[:], xT[0, cb * 128 : (cb + 1) * 128, 0:512])
                    xt_pre[(0, 0, cb)] = xtp

            load_w_chunk(0)
            prefetch_xt(0, 4)
            load_w_chunk(1)
            prefetch_xt(4, 8)
            cos16 = wpool.tile([128, t], f16)
            nc.sync.dma_start(cos16[:], cos2[:, :])
            warm = wpool.tile([128, 1], f32)
            nc.scalar.activation(warm[:], cos16[:, 0:1], EXP, scale=1.0)
            load_w_chunk(2)
            prefetch_xt(8, 12)
            load_w_chunk(3)
            prefetch_xt(12, 16)
            # tables and sin aren't needed until the first attention (~28us);
            # keep them out of the first projection group's xt stream
            mask_sb = wpool.tile([128, 128], f16)
            nc.sync.dma_start(mask_sb[:], maskf[:, :])
            ones_sb = wpool.tile([128, 1], f16)
            nc.sync.dma_start(ones_sb[:], onesv[:, :])
            id_sb = wpool.tile([128, 128], f16)
            nc.sync.dma_start(id_sb[:], ident[:, :])
            sin16 = wpool.tile([128, t], f16)
            nc.sync.dma_start(sin16[:], sin2[:, :])
            # second tile-group's leading x tiles, so its projection doesn't
            # starve while wp and the in-loop stream catch up
            for cb in range(8):
                xtp = wkp.tile([128, 512], f16, tag="xt", bufs=32, name=f"xtq{cb}")
                nc.sync.dma_start(
                    xtp[:], xT[0, cb * 128 : (cb + 1) * 128, 512:1024]
                )
                xt_pre[(0, 1, cb)] = xtp
            # wp is first needed by the output projection at ~40us; its DMA is
            # emitted after the first projection group so it doesn't displace
            # the startup-critical x/weight stream
            wp_sb = wpool.tile([128, HL * c], f16)  # [p, (f, cout)]
            wp_loaded = []

            def load_wp():
                if not wp_loaded:
                    nc.sync.dma_start(
                        wp_sb[:].rearrange("p (f n) -> p f n", n=c),
                        wp.rearrange("(f p) n -> p f n", p=128),
                    )
                    wp_loaded.append(True)

            slots = []  # completed (bi, AT, i4) awaiting output projection

            def get_xt(bi, i4, cb, ts_):
                key = (bi, i4, cb)
                if key in xt_pre:
                    return xt_pre.pop(key)
                xt_ = wkp.tile([128, 512], f16, tag="xt", bufs=32)
                nc.sync.dma_start(xt_[:], xT[bi, cb * 128 : (cb + 1) * 128, ts_])
                return xt_

            def rope_ops(ps_t, dest, ts_, eng=None):
                # dest[:, ts_] = ps*cosI + swap_adjacent(ps)*sinS (host permuted
                # W columns so rotate-half pairs are adjacent partitions).
                # When eng is gpsimd (no stream_shuffle there), the shuffle op
                # is returned separately to be queued on the vector engine; the
                # muls/add run on gpsimd in parallel with the vector queue.
                eng = eng or nc.vector
                ra = wkp.tile([128, 512], f32, tag="ra", bufs=3)
                rb = wkp.tile([128, 512], f32, tag="rb", bufs=3)
                rs = wkp.tile([128, 512], f32, tag="rs", bufs=3)
                shuf = lambda: nc.vector.stream_shuffle(rs[:], ps_t[:], swap_mask)
                rest = [
                    lambda: eng.tensor_mul(ra[:], ps_t[:], cos16[:, ts_]),
                    lambda: eng.tensor_mul(rb[:], rs[:], sin16[:, ts_]),
                    lambda: eng.tensor_add(dest[:, ts_], ra[:], rb[:]),
                ]
                return shuf, rest

            def emit_po_tile(slot, it, n, copy_eng, dma_eng=None):
                """One output-projection tile: 2 accumulated matmuls on the
                's' psum ring, copy to sbuf (given engine), DMA out."""
                sbi, sAT, si4 = slot
                po = pp.tile([128, 512], f32, tag="s", bufs=2, name="po")
                for hh in range(HL):
                    nc.tensor.matmul(
                        po[:],
                        sAT[hh][:, it * 128 : (it + 1) * 128],
                        wp_sb[:, hh * c + n * 512 : hh * c + (n + 1) * 512],
                        start=(hh == 0), stop=(hh == HL - 1),
                    )
                po_sb = wkp.tile([128, 512], f16, tag="yout", bufs=16)
                if copy_eng == "v":
                    nc.vector.tensor_copy(po_sb[:], po[:])
                else:
                    nc.scalar.copy(po_sb[:], po[:])
                (dma_eng or nc.sync).dma_start(
                    y[sbi, it * 128 : (it + 1) * 128, n * 512 : (n + 1) * 512],
                    po_sb[:],
                )

            def po_tiles(slot):
                sbi, sAT, si4 = slot
                return [
                    (slot, si4 * 4 + r, n)
                    for r in range(4)
                    for n in range(c // 512)
                ]

            def emit_po_filler(tiles, vec_fill):
                """PE filler at a phase boundary: alternate copy engines, with
                vec_fill ops interleaved after each vector-side copy."""
                vi = 0
                for ti, (slot, it, n) in enumerate(tiles):
                    emit_po_tile(slot, it, n, "s")
                    if ti % 2 == 0 and vi < len(vec_fill):
                        vec_fill[vi]()
                        vi += 1
                for op in vec_fill[vi:]:
                    op()

            def emit_po_drain(tiles):
                """Kernel-tail drain: nothing else is running, so spread the
                copies and DMAs across engines to shorten the serial tail."""
                for ti, (slot, it, n) in enumerate(tiles):
                    emit_po_tile(
                        slot, it, n,
                        "v" if ti % 2 == 0 else "s",
                        dma_eng=(nc.sync if ti % 2 == 0 else nc.gpsimd),
                    )

            def attn2(i4, QT, AT, KT, Vn, prediag=None, po_queue=None, heads=None):
                """Both local heads in one j-loop; E(j-1) consumed while the
                act engine computes E(j), so the PE matmul stream never waits
                on the exp latency. pav accumulators share the 'proj' psum
                ring (liveness is disjoint from the projection group's)."""
                heads = list(heads if heads is not None else range(HL))
                qs = slice(i4 * 512, (i4 + 1) * 512)
                jmax = njb * (i4 + 1) - 1
                pav = {
                    h: pp.tile([128, 512], f32, tag="proj", bufs=4, name=f"pav{h}")
                    for h in heads
                }
                pden = {
                    h: pp.tile([1, 512], f32, tag=f"den{h}", bufs=1, name=f"pden{h}")
                    for h in heads
                }
                Es = {}

                def consume(h, j, last):
                    E, off = Es.pop((h, j))
                    nc.tensor.matmul(
                        pden[h][:, off:512],
                        ones_sb[:, 0:1],
                        E[:, off:512],
                        start=(j == 0), stop=last,
                        skip_group_check=True,
                    )
                    nc.tensor.matmul(
                        pav[h][:, off:512],
                        Vn[:, j * 128 : (j + 1) * 128],
                        E[:, off:512],
                        start=(j == 0), stop=last,
                        skip_group_check=True,
                    )

                fired = prediag is None
                for j in range(jmax + 1):
                    diag = j - njb * i4
                    if not fired and diag >= 0:
                        prediag()
                        fired = True
                    # the j-loop is act-engine-bound (2 exps ~1.6us vs 1.28us
                    # of PE matmuls per j): one output-proj tile every other j
                    # soaks up the PE slack without outrunning the exp stream.
                    # Placed at the top of the iteration so its psum-ring slot
                    # comes from j-1 (exp already drained), not this j.
                    if j >= 3 and j % 2 == 1 and po_queue:
                        pslot, pit, pn = po_queue.pop(0)
                        emit_po_tile(pslot, pit, pn, "v")
                    off = max(diag, 0) * 128  # skip q cols left of diag
                    for h in heads:
                        pst = pp.tile([128, 512], f32, tag="s", bufs=2, name="pst")
                        nc.tensor.matmul(
                            pst[:, off:512],
                            KT[:, j * 128 : (j + 1) * 128],
                            QT[h][:, i4 * 512 + off : (i4 + 1) * 512],
                            start=True, stop=True,
                        )
                        E = wkp.tile([128, 512], f16, tag="E", bufs=10)
                        nc.scalar.activation(
                            E[:, off:512], pst[:, off:512], EXP, scale=SCALE
                        )
                        if diag >= 0:
                            # zero strictly-lower triangle of the diag block
                            nc.vector.tensor_mul(
                                E[:, off : off + 128], E[:, off : off + 128], mask_sb[:]
                            )
                        Es[(h, j)] = (E, off)
                        if j > 0:
                            consume(h, j - 1, last=False)
                # one more po tile covers the exp(jmax) drain before the
                # final consume pair
                if po_queue:
                    pslot, pit, pn = po_queue.pop(0)
                    emit_po_tile(pslot, pit, pn, "v")
                for h in heads:
                    consume(h, jmax, last=True)
                for h in heads:
                    rec = wkp.tile([1, 512], f32, tag="rec", bufs=2)
                    nc.vector.reciprocal_approx_fast(rec[:], pden[h][:])
                    rbc = wkp.tile([128, 512], f32, tag="rbc", bufs=2)
                    nc.gpsimd.partition_broadcast(rbc[:], rec[:])
                    nc.vector.tensor_mul(AT[h][:, qs], pav[h][:], rbc[:])

            for bi in range(b):
                QT = [
                    dpool.tile([128, t], f16, tag=f"qt{h}", bufs=2, name=f"QT{h}")
                    for h in range(HL)
                ]
                KT = dpool.tile([128, t], f16, tag="kt", bufs=2)
                VT = dpool.tile([128, t], f16, tag="vtt", bufs=2)
                Vn = dpool.tile([128, t], f16, tag="vn", bufs=2)
                AT = [
                    dpool.tile([128, t], f16, tag=f"at{h}", bufs=2, name=f"AT{h}")
                    for h in range(HL)
                ]

                for i4 in range(nt):
                    ts_ = slice(i4 * 512, (i4 + 1) * 512)
                    # ---- QKV projection group ----
                    ps = {
                        kind: pp.tile([128, 512], f32, tag="proj", bufs=4, name=f"ps_{kind}")
                        for kind in ("q0", "q1", "k", "v")
                    }
                    for cb in range(ncb):
                        xt_ = get_xt(bi, i4, cb, ts_)
                        xtr = xt_[:]
                        st, sp = (cb == 0), (cb == ncb - 1)
                        wi, cbl = cb // nw, cb % nw
                        base = cbl * HL * D
                        nc.tensor.matmul(
                            ps["q0"][:], wq_sbs[wi][:, base : base + 128], xtr,
                            start=st, stop=sp,
                        )
                        nc.tensor.matmul(
                            ps["q1"][:], wq_sbs[wi][:, base + 128 : base + 256], xtr,
                            start=st, stop=sp,
                        )
                        nc.tensor.matmul(
                            ps["k"][:], wk_sbs[wi][:, cbl * 128 : (cbl + 1) * 128], xtr,
                            start=st, stop=sp,
                        )
                        nc.tensor.matmul(
                            ps["v"][:], wv_sbs[wi][:, cbl * 128 : (cbl + 1) * 128], xtr,
                            start=st, stop=sp,
                        )

                    load_wp()
                    shuf0, rest0 = rope_ops(ps["q0"], QT[0], ts_)
                    rq0 = [shuf0] + rest0
                    # second head's rope muls/add run on the (otherwise idle)
                    # gpsimd engine, in parallel with the vector queue's
                    # rope(q0)/rope(k). gpsimd cannot read PSUM, so q1 is
                    # staged to SBUF by the act engine first; only the shuffle
                    # needs the vector engine.
                    q1sb = wkp.tile([128, 512], f16, tag="q1s", bufs=2)
                    rs1 = wkp.tile([128, 512], f16, tag="rs1", bufs=2)
                    ra1 = wkp.tile([128, 512], f16, tag="ra1", bufs=2)
                    rb1 = wkp.tile([128, 512], f16, tag="rb1", bufs=2)
                    nc.scalar.copy(q1sb[:], ps["q1"][:])
                    shuf1 = lambda: nc.vector.stream_shuffle(rs1[:], q1sb[:], swap_mask)
                    rest1 = [
                        lambda: nc.gpsimd.tensor_mul(ra1[:], q1sb[:], cos16[:, ts_]),
                        lambda: nc.gpsimd.tensor_mul(rb1[:], rs1[:], sin16[:, ts_]),
                        lambda: nc.gpsimd.tensor_add(QT[1][:, ts_], ra1[:], rb1[:]),
                    ]
                    shufk, restk = rope_ops(ps["k"], KT, ts_)
                    rk = [shufk] + restk
                    vt_op = [lambda: nc.vector.tensor_copy(VT[:, ts_], ps["v"][:])]

                    def prediag(VT=VT, Vn=Vn, i4=i4):
                        for jj in range(4):
                            col = i4 * 512 + jj * 128
                            ptp = pp.tile([128, 128], f16, tag="s", bufs=2, name="pt")
                            nc.tensor.transpose(ptp[:], VT[:, col : col + 128], id_sb[:])
                            nc.vector.tensor_copy(Vn[:, col : col + 128], ptp[:])

                    slot = slots.pop(0) if slots else None
                    # q1's shuffle must be emitted before the gpsimd ops that
                    # read its output (cross-engine deps follow emission order)
                    tiles = po_tiles(slot) if slot is not None else []
                    fill_a, rest = tiles[:8], tiles[8:]

                    if i4 == 0:
                        # first tile of a batch: the whole rope chain would
                        # serialize on the vector queue right before a tiny,
                        # diag-first attention. Route BOTH q-head ropes through
                        # gpsimd (staged via the idle act engine) so vector
                        # only does rope(k), and run the heads serially so
                        # QT1's later arrival hides behind h0's j-loop.
                        q0sb = wkp.tile([128, 512], f16, tag="q0s", bufs=2)
                        rs0 = wkp.tile([128, 512], f16, tag="rs0", bufs=2)
                        ra0 = wkp.tile([128, 512], f16, tag="ra0", bufs=2)
                        rb0 = wkp.tile([128, 512], f16, tag="rb0", bufs=2)
                        nc.scalar.copy(q0sb[:], ps["q0"][:])
                        nc.vector.stream_shuffle(rs0[:], q0sb[:], swap_mask)
                        nc.gpsimd.tensor_mul(ra0[:], q0sb[:], cos16[:, ts_])
                        nc.gpsimd.tensor_mul(rb0[:], rs0[:], sin16[:, ts_])
                        nc.gpsimd.tensor_add(QT[0][:, ts_], ra0[:], rb0[:])
                        shuf1()
                        for op in rest1:  # gpsimd rope(q1) body
                            op()
                        vec_ops = vt_op + rk
                        if slot is not None:
                            emit_po_filler(fill_a, vec_ops)
                        else:
                            for op in vec_ops:
                                op()
                        attn2(i4, QT, AT, KT, Vn, prediag=prediag,
                              po_queue=rest, heads=(0,))
                        attn2(i4, QT, AT, KT, Vn, po_queue=rest, heads=(1,))
                    else:
                        shuf1()
                        for op in rest1:  # gpsimd rope(q1) body
                            op()

                        # boundary filler A: first tiles of the pending output
                        # proj, with this tile's VT copy + rope(q0) + rope(k)
                        # interleaved into the vector queue so attention deps
                        # resolve in time. VT first: the per-i4 transposes
                        # (prediag) depend on it. Remaining tiles interleave
                        # into the attention j-loop; leftovers become filler B,
                        # which covers the rec/rbc/AT drain before the next
                        # projection group reuses the psum ring.
                        vec_ops = vt_op + rq0 + rk
                        if slot is not None:
                            emit_po_filler(fill_a, vec_ops)
                        else:
                            for op in vec_ops:
                                op()

                        attn2(i4, QT, AT, KT, Vn, prediag=prediag, po_queue=rest)

                    if rest:
                        emit_po_filler(rest, [])

                    slots.append((bi, AT, i4))

            # drain the final output-projection slot
            slot = slots.pop(0)
            emit_po_drain(po_tiles(slot))

    nc.compile()
    return nc


def host_inputs(x, Wq, Wk, Wv, Wp, ncores=NCORES, mmdt="f16"):
    import ml_dtypes

    mdt = {
        "f32r": np.float32,
        "bf16": ml_dtypes.bfloat16,
        "f16": np.float16,
    }[mmdt]
    b, t, c = x.shape
    d = D
    xT = np.ascontiguousarray(np.transpose(x, (0, 2, 1)))  # [B, C, T]
    inv = (1.0 / (10000.0 ** (np.arange(0, d, 2, dtype=np.float32) / np.float32(d)))).astype(np.float32)
    pos = np.arange(t, dtype=np.float32)
    fr = np.outer(pos, inv).astype(np.float32)  # [T, 64]
    cosT = np.cos(fr).T.astype(np.float32)  # [64, T]
    sinT = np.sin(fr).T.astype(np.float32)
    # pair-interleaved rope tables: partition 2m,2m+1 <- freq m; sign -/+ on sin
    cosI = np.ascontiguousarray(np.repeat(cosT, 2, axis=0))  # [128, T]
    sinS = np.ascontiguousarray(np.stack([-sinT, sinT], axis=1).reshape(128, t))
    # column permutation putting rope pair (m, m+64) at (2m, 2m+1), per head
    perm = np.stack([np.arange(64), np.arange(64) + 64], 1).reshape(128)
    maskf = np.triu(np.ones((128, 128), np.float32))
    onesv = np.ones((128, 1), np.float32)
    ident = np.eye(128, dtype=np.float32)

    def permute_heads(w):
        nh = w.shape[1] // d
        wv_ = w.reshape(w.shape[0], nh, d)
        return np.ascontiguousarray(wv_[:, :, perm].reshape(w.shape))

    Wq_p = permute_heads(Wq)
    Wk_p = permute_heads(Wk)

    xTm = xT.astype(mdt) if mdt is not np.float32 else xT
    in_maps = []
    for ci in range(ncores):
        qs = slice(ci * HL * d, (ci + 1) * HL * d)
        in_maps.append(
            {
                "xT": xTm,
                "wq": np.ascontiguousarray(Wq_p[:, qs]).astype(mdt),
                "wk": np.ascontiguousarray(Wk_p[:, ci * d : (ci + 1) * d]).astype(mdt),
                "wv": np.ascontiguousarray(Wv[:, ci * d : (ci + 1) * d]).astype(mdt),
                "wp": np.ascontiguousarray(Wp[qs, :]).astype(mdt),
                "cos2": cosI.astype(mdt),
                "sin2": sinS.astype(mdt),
                "maskf": maskf.astype(mdt),
                "onesv": onesv.astype(mdt),
                "ident": ident.astype(mdt),
            }
        )
    return in_maps


_NC_CACHE = {}

MMDT = "f16"


def _get_nc(mmdt=None):
    mmdt = mmdt or MMDT
    key = (B, T, C, mmdt)
    if key not in _NC_CACHE:
        _NC_CACHE[key] = build_nc(B, T, C, mmdt=mmdt)
    return _NC_CACHE[key]


def _install_cc_error_surfacing():
    """Make neuronx_cc hook failures print a real traceback instead of the
    opaque PJRT 'py_result' error."""
    try:
        from concourse import bass2jax

        bass2jax.install_neuronx_cc_hook()
        import libneuronxla

        if getattr(libneuronxla, "_tb_wrapped", False):
            return
        inner = libneuronxla.neuronx_cc

        def wrapped(*a, **k):
            try:
                return inner(*a, **k)
            except BaseException:
                import traceback

                traceback.print_exc()
                raise

        libneuronxla.neuronx_cc = wrapped
        libneuronxla._tb_wrapped = True
    except Exception:
        pass


def run_spmd(x, Wq, Wk, Wv, Wp, trace=False, mmdt=None):
    from concourse.bass_utils import run_bass_kernel_spmd

    mmdt = mmdt or MMDT
    _install_cc_error_surfacing()

    nc = _get_nc(mmdt)
    in_maps = host_inputs(x, Wq, Wk, Wv, Wp, mmdt=mmdt)
    last_err = None
    for attempt in range(3):
        try:
            res = run_bass_kernel_spmd(
                nc, in_maps, core_ids=list(range(NCORES)), trace=trace
            )
            break
        except Exception as e:  # transient NRT device faults: retry
            last_err = e
            import time as _time

            _time.sleep(5.0)
    else:
        raise last_err
    acc = res.results[0]["y"].astype(np.float32)
    for i in range(1, NCORES):
        acc += res.results[i]["y"]
    return acc.astype(np.float32), res


def kernel(x, Wq, Wk, Wv, Wp):
    out, _ = run_spmd(x, Wq, Wk, Wv, Wp, trace=False)
    return out


# revision 61
# speedup vs baseline: 1.0267x; 1.0267x over previous
import sys

if "/opt/trn_rl_repo" not in sys.path:
    sys.path.insert(0, "/opt/trn_rl_repo")

import numpy as np

B, T, C = 2, 2048, 2048
H, H_KV = 16, 8
D = C // H  # 128
NCORES = 8
HL = H // NCORES  # 2 local query heads per core; 1 kv head per core

SCALE = 0.08838834764831845  # 1/sqrt(128)


def build_nc(b=B, t=T, c=C, mmdt="f16"):
    """Per-core Bass program, merged proj/attention/output pipeline.

    Same program on all 8 cores; the sharding lives in the input data each
    core receives (its 2 query heads + 1 kv head slice of Wq/Wk/Wv, and the
    matching 256-row slice of Wp; partial y outputs summed on host)."""
    import concourse.bass as bass  # noqa: F401
    import concourse.mybir as mybir
    import concourse.tile as tile
    from concourse import bacc

    f32 = mybir.dt.float32
    f16 = {
        "f32r": mybir.dt.float32r,
        "bf16": mybir.dt.bfloat16,
        "f16": mybir.dt.float16,
    }[mmdt]
    EXP = mybir.ActivationFunctionType.Exp

    ncb = c // 128  # contraction blocks for projections
    nt = t // 512  # 512-wide t tiles
    njb = 512 // 128  # k-blocks per 512-wide q tile

    nc = bacc.Bacc("TRN2", target_bir_lowering=False, debug=False)

    xT = nc.dram_tensor("xT", [b, c, t], f16, kind="ExternalInput")
    wq = nc.dram_tensor("wq", [c, HL * D], f16, kind="ExternalInput")
    wk = nc.dram_tensor("wk", [c, D], f16, kind="ExternalInput")
    wv = nc.dram_tensor("wv", [c, D], f16, kind="ExternalInput")
    wp = nc.dram_tensor("wp", [HL * D, c], f16, kind="ExternalInput")
    cos2 = nc.dram_tensor("cos2", [128, t], f16, kind="ExternalInput")
    sin2 = nc.dram_tensor("sin2", [128, t], f16, kind="ExternalInput")
    maskf = nc.dram_tensor("maskf", [128, 128], f16, kind="ExternalInput")
    onesv = nc.dram_tensor("onesv", [128, 1], f16, kind="ExternalInput")
    ident = nc.dram_tensor("ident", [128, 128], f16, kind="ExternalInput")
    y = nc.dram_tensor("y", [b, t, c], f16, kind="ExternalOutput")

    swap_mask = [i ^ 1 for i in range(32)]

    with tile.TileContext(nc) as tc:
        with (
            tc.tile_pool(name="wts", bufs=1) as wpool,
            tc.tile_pool(name="data", bufs=1) as dpool,
            tc.tile_pool(name="work", bufs=2) as wkp,
            tc.tile_pool(name="psum", bufs=1, space="PSUM") as pp,
        ):
            # ---- preamble DMA order is the startup critical path (one
            # shared DMA engine): small tables + first weight chunks first,
            # then x-tile prefetch, then the remaining weight chunks. ----
            nw = max(ncb // 4, 1)

            wq_sbs, wk_sbs, wv_sbs = [], [], []

            def load_w_chunk(wi):
                cbs = slice(wi * nw * 128, (wi + 1) * nw * 128)
                wq_i = wpool.tile([128, nw * HL * D], f16, name=f"wq{wi}")
                nc.sync.dma_start(
                    wq_i[:].rearrange("p (cb d) -> p cb d", d=HL * D),
                    wq[cbs, :].rearrange("(cb p) d -> p cb d", p=128),
                )
                wq_sbs.append(wq_i)
                wk_i = wpool.tile([128, nw * D], f16, name=f"wk{wi}")
                nc.sync.dma_start(
                    wk_i[:].rearrange("p (cb d) -> p cb d", d=D),
                    wk[cbs, :].rearrange("(cb p) d -> p cb d", p=128),
                )
                wk_sbs.append(wk_i)
                wv_i = wpool.tile([128, nw * D], f16, name=f"wv{wi}")
                nc.sync.dma_start(
                    wv_i[:].rearrange("p (cb d) -> p cb d", d=D),
                    wv[cbs, :].rearrange("(cb p) d -> p cb d", p=128),
                )
                wv_sbs.append(wv_i)

            xt_pre = {}

            def prefetch_xt(cb0, cb1):
                for cb in range(cb0, cb1):
                    xtp = wkp.tile([128, 512], f16, tag="xt", bufs=32, name=f"xtp{cb}")
                    nc.sync.dma_start(xtp[:], xT[0, cb * 128 : (cb + 1) * 128, 0:512])
                    xt_pre[(0, 0, cb)] = xtp

            load_w_chunk(0)
            prefetch_xt(0, 4)
            load_w_chunk(1)
            prefetch_xt(4, 8)
            cos16 = wpool.tile([128, t], f16)
            nc.sync.dma_start(cos16[:], cos2[:, :])
            warm = wpool.tile([128, 1], f32)
            nc.scalar.activation(warm[:], cos16[:, 0:1], EXP, scale=1.0)
            load_w_chunk(2)
            prefetch_xt(8, 12)
            load_w_chunk(3)
            prefetch_xt(12, 16)
            # tables and sin aren't needed until the first attention (~28us);
            # keep them out of the first projection group's xt stream
            mask_sb = wpool.tile([128, 128], f16)
            nc.sync.dma_start(mask_sb[:], maskf[:, :])
            ones_sb = wpool.tile([128, 1], f16)
            nc.sync.dma_start(ones_sb[:], onesv[:, :])
            id_sb = wpool.tile([128, 128], f16)
            nc.sync.dma_start(id_sb[:], ident[:, :])
            sin16 = wpool.tile([128, t], f16)
            nc.sync.dma_start(sin16[:], sin2[:, :])
            # second tile-group's leading x tiles, so its projection doesn't
            # starve while wp and the in-loop stream catch up
            for cb in range(8):
                xtp = wkp.tile([128, 512], f16, tag="xt", bufs=32, name=f"xtq{cb}")
                nc.sync.dma_start(
                    xtp[:], xT[0, cb * 128 : (cb + 1) * 128, 512:1024]
                )
                xt_pre[(0, 1, cb)] = xtp
            # wp is first needed by the output projection at ~40us; its DMA is
            # emitted after the first projection group so it doesn't displace
            # the startup-critical x/weight stream
            wp_sb = wpool.tile([128, HL * c], f16)  # [p, (f, cout)]
            wp_loaded = []

            def load_wp():
                if not wp_loaded:
                    nc.sync.dma_start(
                        wp_sb[:].rearrange("p (f n) -> p f n", n=c),
                        wp.rearrange("(f p) n -> p f n", p=128),
                    )
                    wp_loaded.append(True)

            slots = []  # completed (bi, AT, i4) awaiting output projection

            def get_xt(bi, i4, cb, ts_):
                key = (bi, i4, cb)
                if key in xt_pre:
                    return xt_pre.pop(key)
                xt_ = wkp.tile([128, 512], f16, tag="xt", bufs=32)
                nc.sync.dma_start(xt_[:], xT[bi, cb * 128 : (cb + 1) * 128, ts_])
                return xt_

            def rope_ops(ps_t, dest, ts_, eng=None):
                # dest[:, ts_] = ps*cosI + swap_adjacent(ps)*sinS (host permuted
                # W columns so rotate-half pairs are adjacent partitions).
                # When eng is gpsimd (no stream_shuffle there), the shuffle op
                # is returned separately to be queued on the vector engine; the
                # muls/add run on gpsimd in parallel with the vector queue.
                eng = eng or nc.vector
                ra = wkp.tile([128, 512], f32, tag="ra", bufs=3)
                rb = wkp.tile([128, 512], f32, tag="rb", bufs=3)
                rs = wkp.tile([128, 512], f32, tag="rs", bufs=3)
                shuf = lambda: nc.vector.stream_shuffle(rs[:], ps_t[:], swap_mask)
                rest = [
                    lambda: eng.tensor_mul(ra[:], ps_t[:], cos16[:, ts_]),
                    lambda: eng.tensor_mul(rb[:], rs[:], sin16[:, ts_]),
                    lambda: eng.tensor_add(dest[:, ts_], ra[:], rb[:]),
                ]
                return shuf, rest

            def emit_po_tile(slot, it, n, copy_eng, dma_eng=None):
                """One output-projection tile: 2 accumulated matmuls on the
                's' psum ring, copy to sbuf (given engine), DMA out."""
                sbi, sAT, si4 = slot
                po = pp.tile([128, 512], f32, tag="s", bufs=2, name="po")
                for hh in range(HL):
                    nc.tensor.matmul(
                        po[:],
                        sAT[hh][:, it * 128 : (it + 1) * 128],
                        wp_sb[:, hh * c + n * 512 : hh * c + (n + 1) * 512],
                        start=(hh == 0), stop=(hh == HL - 1),
                    )
                po_sb = wkp.tile([128, 512], f16, tag="yout", bufs=16)
                if copy_eng == "v":
                    nc.vector.tensor_copy(po_sb[:], po[:])
                else:
                    nc.scalar.copy(po_sb[:], po[:])
                (dma_eng or nc.sync).dma_start(
                    y[sbi, it * 128 : (it + 1) * 128, n * 512 : (n + 1) * 512],
                    po_sb[:],
                )

            def po_tiles(slot):
                sbi, sAT, si4 = slot
                return [
                    (slot, si4 * 4 + r, n)
                    for r in range(4)
                    for n in range(c // 512)
                ]

            def emit_po_filler(tiles, vec_fill):
                """PE filler at a phase boundary: alternate copy engines, with
                vec_fill ops interleaved after each vector-side copy."""
                vi = 0
                for ti, (slot, it, n) in enumerate(tiles):
                    emit_po_tile(slot, it, n, "s")
                    if ti % 2 == 0 and vi < len(vec_fill):
                        vec_fill[vi]()
                        vi += 1
                for op in vec_fill[vi:]:
                    op()

            def emit_po_drain(tiles):
                """Kernel-tail drain: nothing else is running, so spread the
                copies and DMAs across engines to shorten the serial tail."""
                for ti, (slot, it, n) in enumerate(tiles):
                    emit_po_tile(
                        slot, it, n,
                        "v" if ti % 2 == 0 else "s",
                        dma_eng=(nc.sync if ti % 2 == 0 else nc.gpsimd),
                    )

            def attn2(i4, QT, AT, KT, Vn, prediag=None, po_queue=None, heads=None):
                """Both local heads in one j-loop; E(j-1) consumed while the
                act engine computes E(j), so the PE matmul stream never waits
                on the exp latency. pav accumulators share the 'proj' psum
                ring (liveness is disjoint from the projection group's)."""
                heads = list(heads if heads is not None else range(HL))
                qs = slice(i4 * 512, (i4 + 1) * 512)
                jmax = njb * (i4 + 1) - 1
                pav = {
                    h: pp.tile([128, 512], f32, tag="proj", bufs=4, name=f"pav{h}")
                    for h in heads
                }
                pden = {
                    h: pp.tile([1, 512], f32, tag=f"den{h}", bufs=1, name=f"pden{h}")
                    for h in heads
                }
                Es = {}

                def consume(h, j, last):
                    E, off = Es.pop((h, j))
                    nc.tensor.matmul(
                        pden[h][:, off:512],
                        ones_sb[:, 0:1],
                        E[:, off:512],
                        start=(j == 0), stop=last,
                        skip_group_check=True,
                    )
                    nc.tensor.matmul(
                        pav[h][:, off:512],
                        Vn[:, j * 128 : (j + 1) * 128],
                        E[:, off:512],
                        start=(j == 0), stop=last,
                        skip_group_check=True,
                    )

                fired = prediag is None
                for j in range(jmax + 1):
                    diag = j - njb * i4
                    if not fired and diag >= 0:
                        prediag()
                        fired = True
                    # the j-loop is act-engine-bound (2 exps ~1.6us vs 1.28us
                    # of PE matmuls per j): one output-proj tile every other j
                    # soaks up the PE slack without outrunning the exp stream.
                    # Placed at the top of the iteration so its psum-ring slot
                    # comes from j-1 (exp already drained), not this j.
                    if j >= 3 and j % 2 == 1 and po_queue:
                        pslot, pit, pn = po_queue.pop(0)
                        emit_po_tile(pslot, pit, pn, "v")
                    off = max(diag, 0) * 128  # skip q cols left of diag
                    for h in heads:
                        pst = pp.tile([128, 512], f32, tag="s", bufs=2, name="pst")
                        nc.tensor.matmul(
                            pst[:, off:512],
                            KT[:, j * 128 : (j + 1) * 128],
                            QT[h][:, i4 * 512 + off : (i4 + 1) * 512],
                            start=True, stop=True,
                        )
                        E = wkp.tile([128, 512], f16, tag="E", bufs=10)
                        nc.scalar.activation(
                            E[:, off:512], pst[:, off:512], EXP, scale=SCALE
                        )
                        if diag >= 0:
                            # zero strictly-lower triangle of the diag block
                            nc.vector.tensor_mul(
                                E[:, off : off + 128], E[:, off : off + 128], mask_sb[:]
                            )
                        Es[(h, j)] = (E, off)
                        if j > 0:
                            consume(h, j - 1, last=False)
                # one more po tile covers the exp(jmax) drain before the
                # final consume pair
                if po_queue:
                    pslot, pit, pn = po_queue.pop(0)
                    emit_po_tile(pslot, pit, pn, "v")
                for h in heads:
                    consume(h, jmax, last=True)
                for h in heads:
                    rec = wkp.tile([1, 512], f32, tag="rec", bufs=2)
                    nc.vector.reciprocal_approx_fast(rec[:], pden[h][:])
                    rbc = wkp.tile([128, 512], f32, tag="rbc", bufs=2)
                    nc.gpsimd.partition_broadcast(rbc[:], rec[:])
                    nc.vector.tensor_mul(AT[h][:, qs], pav[h][:], rbc[:])

            for bi in range(b):
                QT = [
                    dpool.tile([128, t], f16, tag=f"qt{h}", bufs=2, name=f"QT{h}")
                    for h in range(HL)
                ]
                KT = dpool.tile([128, t], f16, tag="kt", bufs=2)
                VT = dpool.tile([128, t], f16, tag="vtt", bufs=2)
                Vn = dpool.tile([128, t], f16, tag="vn", bufs=2)
                AT = [
                    dpool.tile([128, t], f16, tag=f"at{h}", bufs=2, name=f"AT{h}")
                    for h in range(HL)
                ]

                for i4 in range(nt):
                    ts_ = slice(i4 * 512, (i4 + 1) * 512)
                    # ---- QKV projection group ----
                    ps = {
                        kind: pp.tile([128, 512], f32, tag="proj", bufs=4, name=f"ps_{kind}")
                        for kind in ("q0", "q1", "k", "v")
                    }
                    for cb in range(ncb):
                        xt_ = get_xt(bi, i4, cb, ts_)
                        xtr = xt_[:]
                        st, sp = (cb == 0), (cb == ncb - 1)
                        wi, cbl = cb // nw, cb % nw
                        base = cbl * HL * D
                        nc.tensor.matmul(
                            ps["q0"][:], wq_sbs[wi][:, base : base + 128], xtr,
                            start=st, stop=sp,
                        )
                        nc.tensor.matmul(
                            ps["q1"][:], wq_sbs[wi][:, base + 128 : base + 256], xtr,
                            start=st, stop=sp,
                        )
                        nc.tensor.matmul(
                            ps["k"][:], wk_sbs[wi][:, cbl * 128 : (cbl + 1) * 128], xtr,
                            start=st, stop=sp,
                        )
                        nc.tensor.matmul(
                            ps["v"][:], wv_sbs[wi][:, cbl * 128 : (cbl + 1) * 128], xtr,
                            start=st, stop=sp,
                        )

                    load_wp()
                    shuf0, rest0 = rope_ops(ps["q0"], QT[0], ts_)
                    rq0 = [shuf0] + rest0
                    # second head's rope muls/add run on the (otherwise idle)
                    # gpsimd engine, in parallel with the vector queue's
                    # rope(q0)/rope(k). gpsimd cannot read PSUM, so q1 is
                    # staged to SBUF by the act engine first; only the shuffle
                    # needs the vector engine.
                    q1sb = wkp.tile([128, 512], f16, tag="q1s", bufs=2)
                    rs1 = wkp.tile([128, 512], f16, tag="rs1", bufs=2)
                    ra1 = wkp.tile([128, 512], f16, tag="ra1", bufs=2)
                    rb1 = wkp.tile([128, 512], f16, tag="rb1", bufs=2)
                    nc.scalar.copy(q1sb[:], ps["q1"][:])
                    shuf1 = lambda: nc.vector.stream_shuffle(rs1[:], q1sb[:], swap_mask)
                    rest1 = [
                        lambda: nc.gpsimd.tensor_mul(ra1[:], q1sb[:], cos16[:, ts_]),
                        lambda: nc.gpsimd.tensor_mul(rb1[:], rs1[:], sin16[:, ts_]),
                        lambda: nc.gpsimd.tensor_add(QT[1][:, ts_], ra1[:], rb1[:]),
                    ]
                    shufk, restk = rope_ops(ps["k"], KT, ts_)
                    rk = [shufk] + restk
                    vt_op = [lambda: nc.vector.tensor_copy(VT[:, ts_], ps["v"][:])]

                    def prediag(VT=VT, Vn=Vn, i4=i4):
                        for jj in range(4):
                            col = i4 * 512 + jj * 128
                            ptp = pp.tile([128, 128], f16, tag="s", bufs=2, name="pt")
                            nc.tensor.transpose(ptp[:], VT[:, col : col + 128], id_sb[:])
                            nc.vector.tensor_copy(Vn[:, col : col + 128], ptp[:])

                    slot = slots.pop(0) if slots else None
                    # q1's shuffle must be emitted before the gpsimd ops that
                    # read its output (cross-engine deps follow emission order)
                    tiles = po_tiles(slot) if slot is not None else []
                    fill_a, rest = tiles[:8], tiles[8:]

                    shuf1()
                    for op in rest1:  # gpsimd rope(q1) body
                        op()

                    # boundary filler A: first tiles of the pending output
                    # proj, with this tile's VT copy + rope(q0) + rope(k)
                    # interleaved into the vector queue so attention deps
                    # resolve in time. VT first: the per-i4 transposes
                    # (prediag) depend on it. Remaining tiles interleave into
                    # the attention j-loop; leftovers become filler B, which
                    # covers the rec/rbc/AT drain before the next projection
                    # group reuses the psum ring.
                    vec_ops = vt_op + rq0 + rk
                    if slot is not None:
                        emit_po_filler(fill_a, vec_ops)
                    else:
                        for op in vec_ops:
                            op()

                    attn2(i4, QT, AT, KT, Vn, prediag=prediag, po_queue=rest)

                    if rest:
                        emit_po_filler(rest, [])

                    slots.append((bi, AT, i4))

            # drain the final output-projection slot
            slot = slots.pop(0)
            emit_po_drain(po_tiles(slot))

    nc.compile()
    return nc


def host_inputs(x, Wq, Wk, Wv, Wp, ncores=NCORES, mmdt="f16"):
    import ml_dtypes

    mdt = {
        "f32r": np.float32,
        "bf16": ml_dtypes.bfloat16,
        "f16": np.float16,
    }[mmdt]
    b, t, c = x.shape
    d = D
    xT = np.ascontiguousarray(np.transpose(x, (0, 2, 1)))  # [B, C, T]
    inv = (1.0 / (10000.0 ** (np.arange(0, d, 2, dtype=np.float32) / np.float32(d)))).astype(np.float32)
    pos = np.arange(t, dtype=np.float32)
    fr = np.outer(pos, inv).astype(np.float32)  # [T, 64]
    cosT = np.cos(fr).T.astype(np.float32)  # [64, T]
    sinT = np.sin(fr).T.astype(np.float32)
    # pair-interleaved rope tables: partition 2m,2m+1 <- freq m; sign -/+ on sin
    cosI = np.ascontiguousarray(np.repeat(cosT, 2, axis=0))  # [128, T]
    sinS = np.ascontiguousarray(np.stack([-sinT, sinT], axis=1).reshape(128, t))
    # column permutation putting rope pair (m, m+64) at (2m, 2m+1), per head
    perm = np.stack([np.arange(64), np.arange(64) + 64], 1).reshape(128)
    maskf = np.triu(np.ones((128, 128), np.float32))
    onesv = np.ones((128, 1), np.float32)
    ident = np.eye(128, dtype=np.float32)

    def permute_heads(w):
        nh = w.shape[1] // d
        wv_ = w.reshape(w.shape[0], nh, d)
        return np.ascontiguousarray(wv_[:, :, perm].reshape(w.shape))

    Wq_p = permute_heads(Wq)
    Wk_p = permute_heads(Wk)

    xTm = xT.astype(mdt) if mdt is not np.float32 else xT
    in_maps = []
    for ci in range(ncores):
        qs = slice(ci * HL * d, (ci + 1) * HL * d)
        in_maps.append(
            {
                "xT": xTm,
                "wq": np.ascontiguousarray(Wq_p[:, qs]).astype(mdt),
                "wk": np.ascontiguousarray(Wk_p[:, ci * d : (ci + 1) * d]).astype(mdt),
                "wv": np.ascontiguousarray(Wv[:, ci * d : (ci + 1) * d]).astype(mdt),
                "wp": np.ascontiguousarray(Wp[qs, :]).astype(mdt),
                "cos2": cosI.astype(mdt),
                "sin2": sinS.astype(mdt),
                "maskf": maskf.astype(mdt),
                "onesv": onesv.astype(mdt),
                "ident": ident.astype(mdt),
            }
        )
    return in_maps


_NC_CACHE = {}

MMDT = "f16"


def _get_nc(mmdt=None):
    mmdt = mmdt or MMDT
    key = (B, T, C, mmdt)
    if key not in _NC_CACHE:
        _NC_CACHE[key] = build_nc(B, T, C, mmdt=mmdt)
    return _NC_CACHE[key]


def _install_cc_error_surfacing():
    """Make neuronx_cc hook failures print a real traceback instead of the
    opaque PJRT 'py_result' error."""
    try:
        from concourse import bass2jax

        bass2jax.install_neuronx_cc_hook()
        import libneuronxla

        if getattr(libneuronxla, "_tb_wrapped", False):
            return
        inner = libneuronxla.neuronx_cc

        def wrapped(*a, **k):
            try:
                return inner(*a, **k)
            except BaseException:
                import traceback

                traceback.print_exc()
                raise

        libneuronxla.neuronx_cc = wrapped
        libneuronxla._tb_wrapped = True
    except Exception:
        pass


def run_spmd(x, Wq, Wk, Wv, Wp, trace=False, mmdt=None):
    from concourse.bass_utils import run_bass_kernel_spmd

    mmdt = mmdt or MMDT
    _install_cc_error_surfacing()

    nc = _get_nc(mmdt)
    in_maps = host_inputs(x, Wq, Wk, Wv, Wp, mmdt=mmdt)
    last_err = None
    for attempt in range(3):
        try:
            res = run_bass_kernel_spmd(
                nc, in_maps, core_ids=list(range(NCORES)), trace=trace
            )
            break
        except Exception as e:  # transient NRT device faults: retry
            last_err = e
            import time as _time

            _time.sleep(5.0)
    else:
        raise last_err
    acc = res.results[0]["y"].astype(np.float32)
    for i in range(1, NCORES):
        acc += res.results[i]["y"]
    return acc.astype(np.float32), res


def kernel(x, Wq, Wk, Wv, Wp):
    out, _ = run_spmd(x, Wq, Wk, Wv, Wp, trace=False)
    return out
